# revision 49
# baseline (speedup 1.0000x reference)
"""DagEncoder (MLP + segment_sum) Trainium2 kernel, 8-core SPMD.

Contract: kernel(**inputs) takes the FULL unsharded inputs of
reference.setup_inputs() and returns the FULL [M, E] output.

Strategy (pure data parallelism over DAG segments):
  - 20000 segments split into 8 cores x 2500 segments; each core gets its
    node range. Within a core, segments are split into 2 "streams" so two
    nodes are processed per PE column (feature-major layout, 2x40 features
    stacked on partitions 0..79; optional ones-rows 80/81 carry b1).
  - Host pads every segment to a multiple of B=2 nodes (zero pad). Within
    a 1024-col chunk, cols [0:512) hold node k=1 of block b=jj (the
    second node), cols [512:1024) hold node k=0 of block b=jj-512.
    Core/stream boundaries are chosen to equalize padded-block counts.
  - Device per 1024-col chunk (mm2 lagged LAG chunks behind mm1):
      mm1 (W1 blockdiag)  -> z1 psum [128, 1024]
      h1 = relu(z1): ACT evacuates cols [0:A], DVE cols [A:1024]
      mm2 (W2 blockdiag)  -> z2 psum [128, 1024]
      h2lo = relu(z2[512:1024] (+b2))      (ACT)
      g = max(z2[0:512], -b2) + h2lo       (DVE scalar_tensor_tensor)
    g[j] is the complete 2-node block sum for block j (up to pad/bias
    corrections); written into per-super tiles and DMA'd to HBM as bf16.
  - Host: per-stream per-segment block sums via cumsum over the
    contiguous block range, exact pad/bias corrections (zero when b2==0),
    then @ W3 + b3.
"""

import sys
import types

sys.path.insert(0, "/opt/trn_rl_repo")

import numpy as np
import ml_dtypes

import concourse.bass as bass
import concourse.bacc as bacc
import concourse.mybir as mybir
import concourse.tile as tile
from concourse.bass_utils import run_bass_kernel_spmd

BF16 = ml_dtypes.bfloat16

NCORES = 8
B = 2            # nodes per block (segment padding unit)
FD = 1024        # columns per chunk
KB = FD // B     # blocks per chunk (512)
SUPER = 2048     # DMA super-chunk columns
CPS = SUPER // FD
ACT_H1 = 560     # cols of each chunk's z1 evacuated on ACT (rest on DVE)
LAG = 2          # chunks mm2 trails mm1 by

# Stash of the last run's BassKernelResults for the dev harness.
LAST_RESULT = None


# ----------------------------------------------------------------------------
# Host-side layout
# ----------------------------------------------------------------------------

def _pack_stream(starts, cnts):
    """Sequential B-node blocks for the segments of one stream."""
    nbs = -(-cnts // B)  # ceil, 0 for empty segments
    total = int(nbs.sum())
    seg_of_blk = np.repeat(np.arange(len(cnts)), nbs)
    j_in_seg = np.arange(total) - np.repeat(np.cumsum(nbs) - nbs, nbs)
    blk_src = starts[seg_of_blk] + j_in_seg * B
    blk_cnt = np.minimum(B, cnts[seg_of_blk] - j_in_seg * B)
    return blk_src, blk_cnt, nbs


def _node_src_for_cols(blk_src, blk_cnt, C):
    """node source index per column (-1 = pad).

    chunk q = j//1024, jj = j%1024; block b = q*512 + jj%512;
    k = 1 if jj < 512 else 0."""
    j = np.arange(C, dtype=np.int64)
    jj = j % FD
    b = (j // FD) * KB + (jj % KB)
    k = np.where(jj < KB, 1, 0)
    src = blk_src[b] + k
    src = np.where((blk_src[b] >= 0) & (k < blk_cnt[b]), src, -1)
    return src


def _gather_T(a, src):
    """a[src].T with src == -1 rows zeroed; [a.shape[1], len(src)] bf16."""
    g = a[np.clip(src, 0, a.shape[0] - 1)]
    g[src < 0] = 0
    return np.ascontiguousarray(g.T)


def _split_streams(ptr, lo, hi):
    """Split segments [lo, hi) into two streams balancing padded blocks."""
    cnts = np.diff(ptr)[lo:hi]
    padded = -(-cnts // B)
    cum = np.concatenate([[0], np.cumsum(padded)])
    s = int(np.searchsorted(cum, cum[-1] // 2))
    s = min(max(s, 1), hi - lo - 1)
    return s


def _build_core_inputs(x, h_node, ptr, seg_lo, seg_hi, C, R):
    """xcat [R, C] bf16 and per-stream packing metadata."""
    xcat = np.zeros((R, C), BF16)
    meta = []
    s_split = _split_streams(ptr, seg_lo, seg_hi)
    for st in range(2):
        lo = seg_lo if st == 0 else seg_lo + s_split
        hi = seg_lo + s_split if st == 0 else seg_hi
        starts = ptr[lo:hi].astype(np.int64)
        cnts = np.diff(ptr)[lo:hi].astype(np.int64)
        blk_src, blk_cnt, seg_nb = _pack_stream(starts, cnts)
        nb = len(blk_src)
        assert nb * B <= C, (nb * B, C)
        bs = np.full(C // B, -1, np.int64)
        bc = np.zeros(C // B, np.int64)
        bs[:nb] = blk_src
        bc[:nb] = blk_cnt
        src = _node_src_for_cols(bs, bc, C)
        r0 = 40 * st
        xcat[r0:r0 + 8, :] = _gather_T(x, src)
        xcat[r0 + 8:r0 + 40, :] = _gather_T(h_node, src)
        if R > 80:
            xcat[80 + st, :] = (src >= 0).astype(BF16)
        meta.append(dict(lo=lo, hi=hi, seg_nb=seg_nb, blk_cnt=blk_cnt, nb=nb))
    return xcat, meta


# ----------------------------------------------------------------------------
# Device program
# ----------------------------------------------------------------------------

def _build_device_program(C, R, b2_zero):
    dt = mybir.dt
    AL = mybir.AluOpType
    ACTF = mybir.ActivationFunctionType
    NCH = C // FD           # chunks
    NSC = -(-C // SUPER)    # super-chunks (last may be half)
    A = ACT_H1              # z1 cols evacuated on ACT per chunk

    nc = bacc.Bacc(None, target_bir_lowering=False)

    xcat = nc.dram_tensor("xcat", [R, C], dt.bfloat16, kind="ExternalInput")
    w1 = nc.dram_tensor("w1blk", [R, 128], dt.bfloat16, kind="ExternalInput")
    w2 = nc.dram_tensor("w2blk", [128, 128], dt.bfloat16, kind="ExternalInput")
    b2s = nc.dram_tensor("b2s", [128, 1], dt.float32, kind="ExternalInput")
    nb2s = nc.dram_tensor("nb2s", [128, 1], dt.float32, kind="ExternalInput")
    outT = nc.dram_tensor("outT", [128, C // B], dt.bfloat16,
                          kind="ExternalOutput")

    from contextlib import ExitStack

    with tile.TileContext(nc) as tc, ExitStack() as ctx:
        consts = ctx.enter_context(tc.tile_pool(name="consts", bufs=1))
        xin_pool = ctx.enter_context(tc.tile_pool(name="xin", bufs=6))
        h1_pool = ctx.enter_context(tc.tile_pool(name="h1", bufs=5))
        h2_pool = ctx.enter_context(tc.tile_pool(name="h2", bufs=3))
        out_pool = ctx.enter_context(tc.tile_pool(name="out", bufs=3))
        psum = ctx.enter_context(tc.tile_pool(name="psum", bufs=2, space="PSUM"))

        xts = {}

        def load_super(sc):
            w = min(SUPER, C - sc * SUPER)
            xts[sc] = xin_pool.tile([R, SUPER], dt.bfloat16, tag="xt",
                                    name=f"xt_{sc}")
            if sc < 2:
                # fine-grained early supers so mm1(0) starts ASAP; issue
                # from the (still idle) compute engines' DMA queues so the
                # sync engine's descriptor-gen doesn't serialize the boot
                eng = [nc.gpsimd, nc.scalar, nc.sync, nc.sync]
                for q in range(w // FD):
                    eng[(sc * CPS + q) % 4].dma_start(
                        xts[sc][:, q * FD:(q + 1) * FD],
                        xcat[:, (sc * CPS + q) * FD:(sc * CPS + q + 1) * FD])
            else:
                nc.sync.dma_start(xts[sc][:, 0:w],
                                  xcat[:, sc * SUPER:sc * SUPER + w])

        load_super(0)
        w1t = consts.tile([R, 128], dt.bfloat16)
        nc.sync.dma_start(w1t[:], w1[:])
        w2t = consts.tile([128, 128], dt.bfloat16)
        nc.sync.dma_start(w2t[:], w2[:])
        b2t = consts.tile([128, 1], dt.float32)
        nc.sync.dma_start(b2t[:], b2s[:])
        nb2t = consts.tile([128, 1], dt.float32)
        nc.sync.dma_start(nb2t[:], nb2s[:])

        h1s = {}
        parts = {}

        def stage1(i):
            sc, cq = divmod(i, CPS)
            if cq == 0 and sc > 0:
                load_super(sc)
            xt = xts[sc]
            z1 = psum.tile([128, FD], dt.float32, tag="z1", name=f"z1_{i}")
            nc.tensor.matmul(z1[:, 0:512], w1t[:],
                             xt[:, cq * FD:cq * FD + 512],
                             start=True, stop=True)
            nc.tensor.matmul(z1[:, 512:1024], w1t[:],
                             xt[:, cq * FD + 512:(cq + 1) * FD],
                             start=True, stop=True)
            h1 = h1_pool.tile([128, FD], dt.bfloat16, tag="h1", name=f"h1_{i}")
            nc.scalar.activation(h1[:, 0:A], z1[:, 0:A],
                                 ACTF.Relu, bias=0.0, scale=1.0)
            nc.vector.tensor_scalar(h1[:, A:FD], z1[:, A:FD],
                                    0.0, None, AL.max)
            h1s[i] = h1

        def stage2(i):
            sc, cq = divmod(i, CPS)
            wk = min(CPS, NCH - sc * CPS)   # chunks in this super
            if cq == 0:
                parts[sc] = out_pool.tile([128, CPS * KB], dt.bfloat16,
                                          tag="parts", name=f"p_{sc}")
            h1 = h1s.pop(i)
            z2 = psum.tile([128, FD], dt.float32, tag="z2", name=f"z2_{i}")
            # cols [0:512] = k=1 nodes, [512:1024] = k=0 nodes.
            # k=0 half first: the critical path is mm2b -> h2lo(ACT) ->
            # stt(DVE), so its input should land first.
            nc.tensor.matmul(z2[:, 512:1024], w2t[:], h1[:, 512:1024],
                             start=True, stop=True)
            nc.tensor.matmul(z2[:, 0:512], w2t[:], h1[:, 0:512],
                             start=True, stop=True)
            h2lo = h2_pool.tile([128, KB], dt.bfloat16, tag="h2lo",
                                name=f"h2lo_{i}")
            if b2_zero:
                nc.scalar.activation(h2lo[:], z2[:, KB:FD],
                                     ACTF.Relu, bias=0.0, scale=1.0)
            else:
                nc.scalar.activation(h2lo[:], z2[:, KB:FD],
                                     ACTF.Relu, bias=b2t[:], scale=1.0)
            po = parts[sc]
            # block sums straight into the output tile
            stt_scalar = 0.0 if b2_zero else nb2t[:]
            nc.vector.scalar_tensor_tensor(po[:, cq * KB:(cq + 1) * KB],
                                           z2[:, 0:KB], stt_scalar,
                                           h2lo[:], AL.max, AL.add)
            if sc == NSC - 1:
                # fine-grained tail DMA
                nc.sync.dma_start(
                    outT[:, (sc * CPS + cq) * KB:(sc * CPS + cq + 1) * KB],
                    po[:, cq * KB:(cq + 1) * KB])
                if cq == wk - 1:
                    parts.pop(sc)
            elif cq == wk - 1:
                nc.sync.dma_start(
                    outT[:, sc * CPS * KB:(sc * CPS + wk) * KB],
                    parts.pop(sc)[:, 0:wk * KB])

        for i in range(NCH + LAG):
            if i < NCH:
                stage1(i)
            if i >= LAG:
                stage2(i - LAG)

    nc.finalize()
    return nc


# ----------------------------------------------------------------------------
# Entry point
# ----------------------------------------------------------------------------

def _maybe_install_ntff_hook():
    try:
        import antenv.axon_hooks  # noqa: F401
        return
    except ImportError:
        pass
    try:
        from trn_agent_boot.trn_boot import _ntff_profile_via_ctypes
        hook = _ntff_profile_via_ctypes("/opt/axon/libaxon_pjrt.so")
        mod = types.ModuleType("antenv.axon_hooks")
        mod.get_axon_ntff_profile_hook = lambda: hook
        mod.set_axon_ntff_profile_hook = lambda h: None
        sys.modules["antenv.axon_hooks"] = mod
    except Exception:
        pass


def kernel(x, h_node, W1, b1, W2, b2, W3, b3, ptr):
    global LAST_RESULT
    x = np.asarray(x, np.float32)
    h_node = np.asarray(h_node, np.float32)
    W1 = np.asarray(W1, np.float32)
    W2 = np.asarray(W2, np.float32)
    W3 = np.asarray(W3, np.float32)
    b1 = np.asarray(b1, np.float32)
    b2 = np.asarray(b2, np.float32)
    b3 = np.asarray(b3, np.float32)
    ptr = np.asarray(ptr).astype(np.int64)
    N, F = x.shape
    E = h_node.shape[1]
    H = W1.shape[1]
    M = ptr.shape[0] - 1

    b1_zero = bool(np.all(b1 == 0))
    b2_zero = bool(np.all(b2 == 0))
    R = 80 if b1_zero else 82

    cnts = np.diff(ptr)

    # core boundaries: equalize padded-block counts (contiguous seg ranges)
    pb = -(-cnts // B)
    cum = np.concatenate([[0], np.cumsum(pb)])
    bounds = [0]
    for c in range(1, NCORES):
        t = int(np.searchsorted(cum, cum[-1] * c // NCORES))
        bounds.append(min(max(t, bounds[-1] + 1), M - (NCORES - c)))
    bounds.append(M)

    # common padded column count C: max padded blocks over core-streams
    nb_max = 0
    for c in range(NCORES):
        lo, hi = bounds[c], bounds[c + 1]
        s = _split_streams(ptr, lo, hi)
        for st in range(2):
            l2 = lo if st == 0 else lo + s
            h2_ = lo + s if st == 0 else hi
            nb = int(np.sum(-(-cnts[l2:h2_] // B)))
            nb_max = max(nb_max, nb)
    C = -(-nb_max * B // FD) * FD

    # device weights/constants
    w1blk = np.zeros((R, 128), np.float32)
    w1blk[0:40, 0:64] = W1
    w1blk[40:80, 64:128] = W1
    if R > 80:
        w1blk[80, 0:64] = b1
        w1blk[81, 64:128] = b1
    w2blk = np.zeros((128, 128), np.float32)
    w2blk[0:64, 0:64] = W2
    w2blk[64:128, 64:128] = W2
    b2st = np.concatenate([b2, b2]).reshape(128, 1).astype(np.float32)

    in_maps = []
    metas = []
    for c in range(NCORES):
        xcat, meta = _build_core_inputs(x, h_node, ptr,
                                        bounds[c], bounds[c + 1], C, R)
        in_maps.append({
            "xcat": xcat,
            "w1blk": w1blk.astype(BF16),
            "w2blk": w2blk.astype(BF16),
            "b2s": b2st,
            "nb2s": -b2st,
        })
        metas.append(meta)

    nc = _build_device_program(C, R, b2_zero)
    _maybe_install_ntff_hook()
    res = run_bass_kernel_spmd(nc, in_maps, core_ids=list(range(NCORES)))
    LAST_RESULT = res

    # ---- host assembly ----
    # Pad columns: h1_pad = 0 (ones-row is 0 at pads; z1_pad = 0), so
    # v = W2 @ h1_pad = 0.
    # k=0 slot (ACT h2-form): pad contributes hpad = relu(b2)
    # k=1 slot (DVE g-form): pad contributes gpad = max(0, -b2)
    # real k=1 nodes contribute h2 - b2 (correct by +b2 per real k=1 node).
    hpad = np.maximum(b2, 0.0)              # [H]
    gpad = np.maximum(-b2, 0.0)             # [H]

    out = np.zeros((M, E), np.float32)
    NB = C // B
    for c in range(NCORES):
        raw = res.results[c]["outT"]        # [128, NB] bf16 block sums
        partsum = raw.astype(np.float32)
        for st, m in enumerate(metas[c]):
            lo, hi = m["lo"], m["hi"]
            nb = m["nb"]
            seg_nb = m["seg_nb"]            # blocks per segment
            blk_cnt = m["blk_cnt"]          # real nodes per block
            p = partsum[64 * st:64 * st + 64, :nb].T
            csum = np.concatenate([np.zeros((1, H), np.float32),
                                   np.cumsum(p, axis=0)])
            ends = np.cumsum(seg_nb)
            begs = ends - seg_nb
            segdev = csum[ends] - csum[begs]            # [nsegs, H]
            if not b2_zero:
                nreal_lo = np.minimum(blk_cnt, 1)
                nreal_hi = blk_cnt - nreal_lo
                npad_lo = 1 - nreal_lo
                npad_hi = 1 - nreal_hi
                z = np.zeros(1, np.int64)
                bc = np.concatenate([z, np.cumsum(npad_lo)])
                plo = (bc[ends] - bc[begs]).astype(np.float32)
                bc = np.concatenate([z, np.cumsum(npad_hi)])
                phi = (bc[ends] - bc[begs]).astype(np.float32)
                bc = np.concatenate([z, np.cumsum(nreal_hi)])
                rhi = (bc[ends] - bc[begs]).astype(np.float32)
                segdev = (segdev
                          - plo[:, None] * hpad[None, :]
                          - phi[:, None] * gpad[None, :]
                          + rhi[:, None] * b2[None, :])
            out[lo:hi] = segdev @ W3
    out += cnts[:, None].astype(np.float32) * b3[None, :]
    return out


# revision 50
# speedup vs baseline: 1.0171x; 1.0171x over previous
"""DagEncoder (MLP + segment_sum) Trainium2 kernel, 8-core SPMD.

Contract: kernel(**inputs) takes the FULL unsharded inputs of
reference.setup_inputs() and returns the FULL [M, E] output.

Strategy (pure data parallelism over DAG segments):
  - 20000 segments split into 8 cores x 2500 segments; each core gets its
    node range. Within a core, segments are split into 2 "streams" so two
    nodes are processed per PE column (feature-major layout, 2x40 features
    stacked on partitions 0..79; optional ones-rows 80/81 carry b1).
  - Host pads every segment to a multiple of B=2 nodes (zero pad). Within
    a 1024-col chunk, cols [0:512) hold node k=1 of block b=jj (the
    second node), cols [512:1024) hold node k=0 of block b=jj-512.
    Core/stream boundaries are chosen to equalize padded-block counts.
  - Device per 1024-col chunk (mm2 lagged LAG chunks behind mm1):
      mm1 (W1 blockdiag)  -> z1 psum [128, 1024]
      h1 = relu(z1): ACT evacuates cols [0:A], DVE cols [A:1024]
      mm2 (W2 blockdiag)  -> z2 psum [128, 1024]
      h2lo = relu(z2[512:1024] (+b2))      (ACT)
      g = max(z2[0:512], -b2) + h2lo       (DVE scalar_tensor_tensor)
    g[j] is the complete 2-node block sum for block j (up to pad/bias
    corrections); written into per-super tiles and DMA'd to HBM as bf16.
  - Host: per-stream per-segment block sums via cumsum over the
    contiguous block range, exact pad/bias corrections (zero when b2==0),
    then @ W3 + b3.
"""

import sys
import types

sys.path.insert(0, "/opt/trn_rl_repo")

import numpy as np
import ml_dtypes

import concourse.bass as bass
import concourse.bacc as bacc
import concourse.mybir as mybir
import concourse.tile as tile
from concourse.bass_utils import run_bass_kernel_spmd

BF16 = ml_dtypes.bfloat16

NCORES = 8
B = 2            # nodes per block (segment padding unit)
FD = 1024        # columns per chunk
KB = FD // B     # blocks per chunk (512)
SUPER = 2048     # DMA super-chunk columns
CPS = SUPER // FD
ACT_H1 = 560     # cols of each chunk's z1 evacuated on ACT (rest on DVE)
LAG = 2          # chunks mm2 trails mm1 by

# Stash of the last run's BassKernelResults for the dev harness.
LAST_RESULT = None


# ----------------------------------------------------------------------------
# Host-side layout
# ----------------------------------------------------------------------------

def _pack_stream(starts, cnts):
    """Sequential B-node blocks for the segments of one stream."""
    nbs = -(-cnts // B)  # ceil, 0 for empty segments
    total = int(nbs.sum())
    seg_of_blk = np.repeat(np.arange(len(cnts)), nbs)
    j_in_seg = np.arange(total) - np.repeat(np.cumsum(nbs) - nbs, nbs)
    blk_src = starts[seg_of_blk] + j_in_seg * B
    blk_cnt = np.minimum(B, cnts[seg_of_blk] - j_in_seg * B)
    return blk_src, blk_cnt, nbs


def _node_src_for_cols(blk_src, blk_cnt, C):
    """node source index per column (-1 = pad).

    chunk q = j//1024, jj = j%1024; block b = q*512 + jj%512;
    k = 1 if jj < 512 else 0."""
    j = np.arange(C, dtype=np.int64)
    jj = j % FD
    b = (j // FD) * KB + (jj % KB)
    k = np.where(jj < KB, 1, 0)
    src = blk_src[b] + k
    src = np.where((blk_src[b] >= 0) & (k < blk_cnt[b]), src, -1)
    return src


def _gather_T(a, src):
    """a[src].T with src == -1 rows zeroed; [a.shape[1], len(src)] bf16."""
    g = a[np.clip(src, 0, a.shape[0] - 1)]
    g[src < 0] = 0
    return np.ascontiguousarray(g.T)


def _split_streams(ptr, lo, hi):
    """Split segments [lo, hi) into two streams balancing padded blocks."""
    cnts = np.diff(ptr)[lo:hi]
    padded = -(-cnts // B)
    cum = np.concatenate([[0], np.cumsum(padded)])
    s = int(np.searchsorted(cum, cum[-1] // 2))
    s = min(max(s, 1), hi - lo - 1)
    return s


def _build_core_inputs(x, h_node, ptr, seg_lo, seg_hi, C, R):
    """xcat [R, C] bf16 and per-stream packing metadata."""
    xcat = np.zeros((R, C), BF16)
    meta = []
    s_split = _split_streams(ptr, seg_lo, seg_hi)
    for st in range(2):
        lo = seg_lo if st == 0 else seg_lo + s_split
        hi = seg_lo + s_split if st == 0 else seg_hi
        starts = ptr[lo:hi].astype(np.int64)
        cnts = np.diff(ptr)[lo:hi].astype(np.int64)
        blk_src, blk_cnt, seg_nb = _pack_stream(starts, cnts)
        nb = len(blk_src)
        assert nb * B <= C, (nb * B, C)
        bs = np.full(C // B, -1, np.int64)
        bc = np.zeros(C // B, np.int64)
        bs[:nb] = blk_src
        bc[:nb] = blk_cnt
        src = _node_src_for_cols(bs, bc, C)
        r0 = 40 * st
        xcat[r0:r0 + 8, :] = _gather_T(x, src)
        xcat[r0 + 8:r0 + 40, :] = _gather_T(h_node, src)
        if R > 80:
            xcat[80 + st, :] = (src >= 0).astype(BF16)
        meta.append(dict(lo=lo, hi=hi, seg_nb=seg_nb, blk_cnt=blk_cnt, nb=nb))
    return xcat, meta


# ----------------------------------------------------------------------------
# Device program
# ----------------------------------------------------------------------------

def _build_device_program(C, R, b2_zero):
    dt = mybir.dt
    AL = mybir.AluOpType
    ACTF = mybir.ActivationFunctionType
    NCH = C // FD           # chunks
    NSC = -(-C // SUPER)    # super-chunks (last may be half)
    A = ACT_H1              # z1 cols evacuated on ACT per chunk

    nc = bacc.Bacc(None, target_bir_lowering=False)

    xcat = nc.dram_tensor("xcat", [R, C], dt.bfloat16, kind="ExternalInput")
    w1 = nc.dram_tensor("w1blk", [R, 128], dt.bfloat16, kind="ExternalInput")
    w2 = nc.dram_tensor("w2blk", [128, 128], dt.bfloat16, kind="ExternalInput")
    b2s = nc.dram_tensor("b2s", [128, 1], dt.float32, kind="ExternalInput")
    nb2s = nc.dram_tensor("nb2s", [128, 1], dt.float32, kind="ExternalInput")
    outT = nc.dram_tensor("outT", [128, C // B], dt.bfloat16,
                          kind="ExternalOutput")

    from contextlib import ExitStack

    with tile.TileContext(nc) as tc, ExitStack() as ctx:
        consts = ctx.enter_context(tc.tile_pool(name="consts", bufs=1))
        xin_pool = ctx.enter_context(tc.tile_pool(name="xin", bufs=6))
        h1_pool = ctx.enter_context(tc.tile_pool(name="h1", bufs=5))
        h2_pool = ctx.enter_context(tc.tile_pool(name="h2", bufs=3))
        out_pool = ctx.enter_context(tc.tile_pool(name="out", bufs=4))
        psum = ctx.enter_context(tc.tile_pool(name="psum", bufs=2, space="PSUM"))

        xts = {}

        def load_super(sc):
            w = min(SUPER, C - sc * SUPER)
            xts[sc] = xin_pool.tile([R, SUPER], dt.bfloat16, tag="xt",
                                    name=f"xt_{sc}")
            if sc < 2:
                # fine-grained early supers so mm1(0) starts ASAP; issue
                # from the (still idle) compute engines' DMA queues so the
                # sync engine's descriptor-gen doesn't serialize the boot
                eng = [nc.gpsimd, nc.scalar, nc.sync, nc.sync]
                for q in range(w // FD):
                    eng[(sc * CPS + q) % 4].dma_start(
                        xts[sc][:, q * FD:(q + 1) * FD],
                        xcat[:, (sc * CPS + q) * FD:(sc * CPS + q + 1) * FD])
            else:
                nc.sync.dma_start(xts[sc][:, 0:w],
                                  xcat[:, sc * SUPER:sc * SUPER + w])

        load_super(0)
        w1t = consts.tile([R, 128], dt.bfloat16)
        nc.sync.dma_start(w1t[:], w1[:])
        w2t = consts.tile([128, 128], dt.bfloat16)
        nc.sync.dma_start(w2t[:], w2[:])
        b2t = consts.tile([128, 1], dt.float32)
        nc.sync.dma_start(b2t[:], b2s[:])
        nb2t = consts.tile([128, 1], dt.float32)
        nc.sync.dma_start(nb2t[:], nb2s[:])

        h1s = {}
        parts = {}

        def stage1(i):
            sc, cq = divmod(i, CPS)
            if cq == 0 and sc > 0:
                load_super(sc)
            xt = xts[sc]
            z1 = psum.tile([128, FD], dt.float32, tag="z1", name=f"z1_{i}")
            nc.tensor.matmul(z1[:, 0:512], w1t[:],
                             xt[:, cq * FD:cq * FD + 512],
                             start=True, stop=True)
            nc.tensor.matmul(z1[:, 512:1024], w1t[:],
                             xt[:, cq * FD + 512:(cq + 1) * FD],
                             start=True, stop=True)
            h1 = h1_pool.tile([128, FD], dt.bfloat16, tag="h1", name=f"h1_{i}")
            nc.scalar.activation(h1[:, 0:A], z1[:, 0:A],
                                 ACTF.Relu, bias=0.0, scale=1.0)
            nc.vector.tensor_scalar(h1[:, A:FD], z1[:, A:FD],
                                    0.0, None, AL.max)
            h1s[i] = h1

        def stage2(i):
            sc, cq = divmod(i, CPS)
            wk = min(CPS, NCH - sc * CPS)   # chunks in this super
            if cq == 0:
                parts[sc] = out_pool.tile([128, CPS * KB], dt.bfloat16,
                                          tag="parts", name=f"p_{sc}")
            h1 = h1s.pop(i)
            z2 = psum.tile([128, FD], dt.float32, tag="z2", name=f"z2_{i}")
            # cols [0:512] = k=1 nodes, [512:1024] = k=0 nodes.
            # k=0 half first: the critical path is mm2b -> h2lo(ACT) ->
            # stt(DVE), so its input should land first.
            nc.tensor.matmul(z2[:, 512:1024], w2t[:], h1[:, 512:1024],
                             start=True, stop=True)
            nc.tensor.matmul(z2[:, 0:512], w2t[:], h1[:, 0:512],
                             start=True, stop=True)
            h2lo = h2_pool.tile([128, KB], dt.bfloat16, tag="h2lo",
                                name=f"h2lo_{i}")
            if b2_zero:
                nc.scalar.activation(h2lo[:], z2[:, KB:FD],
                                     ACTF.Relu, bias=0.0, scale=1.0)
            else:
                nc.scalar.activation(h2lo[:], z2[:, KB:FD],
                                     ACTF.Relu, bias=b2t[:], scale=1.0)
            po = parts[sc]
            # block sums straight into the output tile
            stt_scalar = 0.0 if b2_zero else nb2t[:]
            nc.vector.scalar_tensor_tensor(po[:, cq * KB:(cq + 1) * KB],
                                           z2[:, 0:KB], stt_scalar,
                                           h2lo[:], AL.max, AL.add)
            if sc == NSC - 1:
                # fine-grained tail DMA
                nc.sync.dma_start(
                    outT[:, (sc * CPS + cq) * KB:(sc * CPS + cq + 1) * KB],
                    po[:, cq * KB:(cq + 1) * KB])
                if cq == wk - 1:
                    parts.pop(sc)
            elif cq == wk - 1:
                nc.sync.dma_start(
                    outT[:, sc * CPS * KB:(sc * CPS + wk) * KB],
                    parts.pop(sc)[:, 0:wk * KB])

        for i in range(NCH + LAG):
            if i < NCH:
                stage1(i)
            if i >= LAG:
                stage2(i - LAG)

    nc.finalize()
    return nc


# ----------------------------------------------------------------------------
# Entry point
# ----------------------------------------------------------------------------

def _maybe_install_ntff_hook():
    try:
        import antenv.axon_hooks  # noqa: F401
        return
    except ImportError:
        pass
    try:
        from trn_agent_boot.trn_boot import _ntff_profile_via_ctypes
        hook = _ntff_profile_via_ctypes("/opt/axon/libaxon_pjrt.so")
        mod = types.ModuleType("antenv.axon_hooks")
        mod.get_axon_ntff_profile_hook = lambda: hook
        mod.set_axon_ntff_profile_hook = lambda h: None
        sys.modules["antenv.axon_hooks"] = mod
    except Exception:
        pass


def kernel(x, h_node, W1, b1, W2, b2, W3, b3, ptr):
    global LAST_RESULT
    x = np.asarray(x, np.float32)
    h_node = np.asarray(h_node, np.float32)
    W1 = np.asarray(W1, np.float32)
    W2 = np.asarray(W2, np.float32)
    W3 = np.asarray(W3, np.float32)
    b1 = np.asarray(b1, np.float32)
    b2 = np.asarray(b2, np.float32)
    b3 = np.asarray(b3, np.float32)
    ptr = np.asarray(ptr).astype(np.int64)
    N, F = x.shape
    E = h_node.shape[1]
    H = W1.shape[1]
    M = ptr.shape[0] - 1

    b1_zero = bool(np.all(b1 == 0))
    b2_zero = bool(np.all(b2 == 0))
    R = 80 if b1_zero else 82

    cnts = np.diff(ptr)

    # core boundaries: equalize padded-block counts (contiguous seg ranges)
    pb = -(-cnts // B)
    cum = np.concatenate([[0], np.cumsum(pb)])
    bounds = [0]
    for c in range(1, NCORES):
        t = int(np.searchsorted(cum, cum[-1] * c // NCORES))
        bounds.append(min(max(t, bounds[-1] + 1), M - (NCORES - c)))
    bounds.append(M)

    # common padded column count C: max padded blocks over core-streams
    nb_max = 0
    for c in range(NCORES):
        lo, hi = bounds[c], bounds[c + 1]
        s = _split_streams(ptr, lo, hi)
        for st in range(2):
            l2 = lo if st == 0 else lo + s
            h2_ = lo + s if st == 0 else hi
            nb = int(np.sum(-(-cnts[l2:h2_] // B)))
            nb_max = max(nb_max, nb)
    C = -(-nb_max * B // FD) * FD

    # device weights/constants
    w1blk = np.zeros((R, 128), np.float32)
    w1blk[0:40, 0:64] = W1
    w1blk[40:80, 64:128] = W1
    if R > 80:
        w1blk[80, 0:64] = b1
        w1blk[81, 64:128] = b1
    w2blk = np.zeros((128, 128), np.float32)
    w2blk[0:64, 0:64] = W2
    w2blk[64:128, 64:128] = W2
    b2st = np.concatenate([b2, b2]).reshape(128, 1).astype(np.float32)

    in_maps = []
    metas = []
    for c in range(NCORES):
        xcat, meta = _build_core_inputs(x, h_node, ptr,
                                        bounds[c], bounds[c + 1], C, R)
        in_maps.append({
            "xcat": xcat,
            "w1blk": w1blk.astype(BF16),
            "w2blk": w2blk.astype(BF16),
            "b2s": b2st,
            "nb2s": -b2st,
        })
        metas.append(meta)

    nc = _build_device_program(C, R, b2_zero)
    _maybe_install_ntff_hook()
    res = run_bass_kernel_spmd(nc, in_maps, core_ids=list(range(NCORES)))
    LAST_RESULT = res

    # ---- host assembly ----
    # Pad columns: h1_pad = 0 (ones-row is 0 at pads; z1_pad = 0), so
    # v = W2 @ h1_pad = 0.
    # k=0 slot (ACT h2-form): pad contributes hpad = relu(b2)
    # k=1 slot (DVE g-form): pad contributes gpad = max(0, -b2)
    # real k=1 nodes contribute h2 - b2 (correct by +b2 per real k=1 node).
    hpad = np.maximum(b2, 0.0)              # [H]
    gpad = np.maximum(-b2, 0.0)             # [H]

    out = np.zeros((M, E), np.float32)
    NB = C // B
    for c in range(NCORES):
        raw = res.results[c]["outT"]        # [128, NB] bf16 block sums
        partsum = raw.astype(np.float32)
        for st, m in enumerate(metas[c]):
            lo, hi = m["lo"], m["hi"]
            nb = m["nb"]
            seg_nb = m["seg_nb"]            # blocks per segment
            blk_cnt = m["blk_cnt"]          # real nodes per block
            p = partsum[64 * st:64 * st + 64, :nb].T
            csum = np.concatenate([np.zeros((1, H), np.float32),
                                   np.cumsum(p, axis=0)])
            ends = np.cumsum(seg_nb)
            begs = ends - seg_nb
            segdev = csum[ends] - csum[begs]            # [nsegs, H]
            if not b2_zero:
                nreal_lo = np.minimum(blk_cnt, 1)
                nreal_hi = blk_cnt - nreal_lo
                npad_lo = 1 - nreal_lo
                npad_hi = 1 - nreal_hi
                z = np.zeros(1, np.int64)
                bc = np.concatenate([z, np.cumsum(npad_lo)])
                plo = (bc[ends] - bc[begs]).astype(np.float32)
                bc = np.concatenate([z, np.cumsum(npad_hi)])
                phi = (bc[ends] - bc[begs]).astype(np.float32)
                bc = np.concatenate([z, np.cumsum(nreal_hi)])
                rhi = (bc[ends] - bc[begs]).astype(np.float32)
                segdev = (segdev
                          - plo[:, None] * hpad[None, :]
                          - phi[:, None] * gpad[None, :]
                          + rhi[:, None] * b2[None, :])
            out[lo:hi] = segdev @ W3
    out += cnts[:, None].astype(np.float32) * b3[None, :]
    return out
